# revision 101
# baseline (speedup 1.0000x reference)
"""Multi-head attention (B=2, S=2048, D=1024, H=16, K=64) on 8 TRN2 cores.

Sharding: core c -> batch b=c//4, head-group g=c%4 (4 heads, 256-wide slice
of Wq/Wk/Wv columns and Wo rows).  Each core computes a partial (2048, 1024)
output in bf16; host sums groups of 4 cores in f32 and adds bo.

Per-core layout (all transposed so no on-chip transposes are needed):
  - host supplies xT = x[b].T  (D, S), bf16
  - Q^T, K^T computed as [gw_col, S] via lhsT=W chunk, rhs=xT chunk
  - scores^T[j, i] via lhsT=K^T chunk, rhs=Q^T into double-buffered 2-bank
    PSUM tiles; one ScalarE Exp covers 1024 elements
  - softmax denominator via a ones column appended to V (V_aug); probs are
    exp(scores/8) with no max subtraction (scores ~N(0,1), no overflow)
  - O^T = V_aug^T @ probs^T; division by the denominator happens on VectorE
    with head-pair batching (one reciprocal per pair, broadcast matmuls to
    the two column groups of one PSUM bank)

Scheduling: the loop is rate-limited by the ScalarE Exp (~1.33us per
j-chunk vs ~0.7us of PE work), so ALL projection work (K, V, Q) plus the
Wo output matmuls of finished i-groups run as due-date-scheduled filler
units inside the attention loop.  The serial projection preamble is
reduced to the two units the first scores need (K(c4=0,m=0), Q(0,m=0));
everything else streams in under the Exp shadow, gated so the in-order
PE queue never head-of-line blocks: a unit is emitted at least one slot
before the first consumer of its output, and AV matmuls are only popped
once their V chunk has been emitted.

All matmul operands are bf16 (PSUM accumulation stays fp32).
"""

import os
import sys
from contextlib import ExitStack

import numpy as np

if "/opt/trn_rl_repo" not in sys.path:
    sys.path.insert(0, "/opt/trn_rl_repo")

import concourse.bass as bass
import concourse.mybir as mybir
import concourse.tile as tile
from concourse import bacc
from concourse.bass import ds, ts
from concourse.bass_utils import run_bass_kernel_spmd

B, S, D = 2, 2048, 1024
H, KS = 16, 64
NCORES = 8
HPC = H // 4          # 4 heads per core
GW = HPC * KS         # 256-wide head-group slice
P = 128
ND = D // P           # 8 contraction chunks over d_model
NM = GW // P          # 2 col chunks of the group slice
NI = 4                # i-groups
IT = S // NI          # 512 rows per i-group
NJ = S // P           # 16 j-chunks
NJJ = 2               # heads per Exp batch ([128,1024] ACT, 2 PSUM banks)
NO = D // 512         # 2 out-col groups for Wo

F32 = mybir.dt.float32
BF16 = mybir.dt.bfloat16
MMDT = BF16
EXP = mybir.ActivationFunctionType.Exp

# estimated PE-stream ns for filler budget accounting
MM512 = 230
MM256 = 120


def _mha_core(tc, out, xT, wkq, wv, wo, bq, bk, bv):
    nc = tc.nc
    with ExitStack() as ctx:
        cp = ctx.enter_context(tc.tile_pool(name="const", bufs=1))
        probs_pool = ctx.enter_context(tc.tile_pool(name="probs", bufs=12))
        out_pool = ctx.enter_context(tc.tile_pool(name="outsb", bufs=3))
        den_pool = ctx.enter_context(tc.tile_pool(name="den", bufs=3))

        # ---- ACT table preload: tiny exp before anything else on ScalarE ----
        warm = cp.tile([1, 16], F32)
        nc.vector.memset(warm[:], 0.0)
        nc.scalar.activation(warm[:], warm[:], EXP)

        # HAM warmup operands: junk matmuls keep the PE busy while DMAs
        # land so real matmuls run at 2.4 GHz instead of the cold 1.2 GHz
        wu_l = cp.tile([P, P], MMDT)
        wu_r = cp.tile([P, IT], MMDT)
        nc.vector.memset(wu_l[:], 0.0)
        nc.vector.memset(wu_r[:], 0.0)

        # ---- inputs to SBUF, chunked in need order across both HW DMA
        # rings (sync ~= scalar ~= 125 GB/s each); biases + wo ride the
        # idle gpsimd SWDGE queue.  Chunks are dc-slices so the
        # projection filler units can consume them as they land.
        # wk | wq | xc0 ship as ONE host-concatenated 2 MB tensor in a
        # single full-rate DMA -- everything the loop start needs
        wkq_sb = cp.tile([P, 2 * ND * GW + ND * IT], MMDT)
        wk_sb = wkq_sb[:, ds(0, ND * GW)].rearrange(
            "p (nd n) -> p nd n", n=GW)
        wq_sb = wkq_sb[:, ds(ND * GW, ND * GW)].rearrange(
            "p (nd n) -> p nd n", n=GW)
        xT_sb = [wkq_sb[:, ds(2 * ND * GW, ND * IT)].rearrange(
            "p (nd s) -> p nd s", s=IT)]
        for c4 in range(1, 4):
            xc = cp.tile([P, ND, IT], MMDT, name=f"xc{c4}")
            xT_sb.append(xc)
        wv_sb = cp.tile([P, ND, GW], MMDT)
        wo_sb = cp.tile([P, NM, D], MMDT)
        bq_sb = cp.tile([P, NM], F32)
        bk_sb = cp.tile([P, NM], F32)
        bv_bc = cp.tile([P, GW], F32)
        # DMA plan.  All inputs are pre-arranged on the host into their
        # exact SBUF layouts, so every transfer is fully contiguous
        # (4-8 KB per partition line) and runs at full HBM rate instead
        # of the ~30% descriptor-bound rate of strided patterns.  The
        # sync ring carries everything the PE consumes, in need order;
        # gpsimd SWDGE takes the rest; the scalar queue stays pure exp.
        nc.gpsimd.dma_start(bq_sb[:], bq[:, :])
        nc.gpsimd.dma_start(bk_sb[:], bk[:, :])
        nc.gpsimd.dma_start(bv_bc[:], bv.partition_broadcast(P))
        nc.sync.dma_start(wkq_sb[:], wkq[:, :])
        CW = ND * IT

        def xchunk(eng, c4, h):
            eng.dma_start(
                xT_sb[c4][:, ds(4 * h, 4), :],
                xT[:, ds(c4 * CW + 4 * h * IT, 4 * IT)].rearrange(
                    "p (nd s) -> p nd s", s=IT))

        nc.gpsimd.dma_start(wv_sb[:], wv.rearrange("p (nd n) -> p nd n", n=GW))
        xchunk(nc.sync, 1, 0)
        xchunk(nc.sync, 1, 1)
        xchunk(nc.sync, 2, 0)
        xchunk(nc.sync, 2, 1)
        xchunk(nc.sync, 3, 0)
        xchunk(nc.sync, 3, 1)
        nc.gpsimd.dma_start(wo_sb[:], wo.rearrange("p (nm n) -> p nm n", n=D))

        QT = cp.tile([P, NM, S], MMDT)
        KT = cp.tile([P, NM, S], MMDT)
        OT = cp.tile([P, NM, S], MMDT)
        # f32 stage for the last i-group's hc=0 Wo partials (filled
        # in-loop once chain(ig3, m=0) lands; consumed in the tail)
        stage3 = cp.tile([P, NI, D], F32)
        # V_aug[:, h, jt, 0:64] = V rows, [:, h, jt, 64] = 1.0 (denominator col)
        V_aug = cp.tile([P, HPC, NJ, KS + 1], MMDT)
        nc.vector.memset(
            V_aug[:, :, :, ds(KS, 1)].rearrange("p h j o -> p (h j o)"), 1.0)

        # ones row for the denominator broadcast matmul (1/den -> 64 rows)
        ones64 = cp.tile([1, KS], MMDT)
        nc.vector.memset(ones64[:], 1.0)

        # PSUM budget (8 banks): ps_s = 2x2, ps_o = 2, ps_m (fillers/Wo/
        # den-broadcast) = 2 shared slots
        with tc.tile_pool(name="ps_s", bufs=2, space="PSUM") as ps_s, \
             tc.tile_pool(name="ps_o", bufs=2, space="PSUM") as ps_o, \
             tc.tile_pool(name="ps_m", bufs=2, space="PSUM") as ps_m:

            # ---- filler units ----------------------------------------
            def k_subunits(c4, m):
                """K^T projection of (i-chunk c4, col chunk m): 4 units of
                2 accumulating matmuls; last adds bias into KT."""
                state = {}

                def unit(k):
                    def emit():
                        if k == 0:
                            state["ps"] = ps_m.tile([P, IT], F32,
                                                    name="kt_ps", tag="m")
                        kt_ps = state["ps"]
                        for dc in (2 * k, 2 * k + 1):
                            nc.tensor.matmul(
                                kt_ps[:],
                                wk_sb[:, dc, ts(m, P)],
                                xT_sb[c4][:, dc, :],
                                start=(dc == 0), stop=(dc == ND - 1),
                            )
                        if k == 3:
                            nc.vector.tensor_scalar_add(
                                KT[:, m, ts(c4, IT)], kt_ps[:],
                                bk_sb[:, ds(m, 1)])
                    return emit
                return [unit(k) for k in range(4)]

            def q_subunits(g, m):
                state = {}

                def unit(k):
                    def emit():
                        if k == 0:
                            state["ps"] = ps_m.tile([P, IT], F32,
                                                    name="qt_ps", tag="m")
                        qt_ps = state["ps"]
                        for dc in (2 * k, 2 * k + 1):
                            nc.tensor.matmul(
                                qt_ps[:],
                                wq_sb[:, dc, ts(m, P)],
                                xT_sb[g][:, dc, :],
                                start=(dc == 0), stop=(dc == ND - 1),
                            )
                        if k == 3:
                            nc.vector.tensor_scalar_add(
                                QT[:, m, ts(g, IT)], qt_ps[:],
                                bq_sb[:, ds(m, 1)])
                    return emit
                return [unit(k) for k in range(4)]

            v_done = [False] * NJ

            def v_subunits(jt):
                """V rows for j-chunk jt: 4 units of 2 accumulating n=256
                matmuls; last adds bias into V_aug and marks v_done."""
                state = {}

                def unit(k):
                    def emit():
                        if k == 0:
                            state["ps"] = ps_m.tile([P, IT], F32,
                                                    name="v_ps", tag="m")
                        v_ps = state["ps"]
                        for dc in (2 * k, 2 * k + 1):
                            nc.tensor.matmul(
                                v_ps[:, 0:GW],
                                xT_sb[jt // 4][:, dc, ts(jt % 4, P)],
                                wv_sb[:, dc, :],
                                start=(dc == 0), stop=(dc == ND - 1),
                            )
                        if k == 3:
                            nc.vector.tensor_add(
                                V_aug[:, :, jt, 0:KS],
                                v_ps[:, 0:GW].rearrange(
                                    "p (h k) -> p h k", h=HPC),
                                bv_bc[:].rearrange("p (h k) -> p h k", h=HPC),
                            )
                            v_done[jt] = True
                    return emit
                return [unit(k) for k in range(4)]

            stage, stage_left = {}, {}

            def wo_unit(it, ncol):
                def emit():
                    g = it // NI
                    if g not in stage and g != NI - 1:
                        stage[g] = out_pool.tile([P, NI, D], MMDT,
                                                 name="ostage", tag="ost",
                                                 bufs=2)
                        stage_left[g] = NI * NO
                    w_ps = ps_m.tile([P, 512], F32, tag="m")
                    for hc in range(NM):
                        nc.tensor.matmul(
                            w_ps[:],
                            OT[:, hc, ts(it, P)],
                            wo_sb[:, hc, ts(ncol, 512)],
                            start=(hc == 0), stop=(hc == NM - 1),
                        )
                    if g == NI - 1:
                        o_sb = out_pool.tile([P, 512], MMDT)
                        nc.vector.tensor_copy(o_sb[:], w_ps[:])
                        eng = nc.sync if (it + ncol) % 2 == 0 else nc.gpsimd
                        eng.dma_start(
                            out[ts(it, P), ts(ncol, 512)], o_sb[:])
                        return
                    st = stage[g]
                    nc.vector.tensor_copy(st[:, it % NI, ts(ncol, 512)],
                                          w_ps[:])
                    stage_left[g] -= 1
                    if stage_left[g] == 0:
                        eng = nc.sync if g % 2 == 0 else nc.gpsimd
                        eng.dma_start(
                            out[ts(g, 4 * P), :].rearrange(
                                "(itl p) d -> p itl d", p=P),
                            st[:])
                        del stage[g]
                return emit

            def den_chain(ig, m, oe_sb, oo_sb, recip2b):
                """Normalize heads 2m, 2m+1 of i-group ig from their SBUF
                evacuations (o rows in oe_sb/oo_sb, bf16 reciprocal of the
                denominators in recip2b -- computed back at evacuation time
                so this stage never waits on the slow 1-partition recip)."""
                bc_ps = ps_m.tile([P, IT], F32, tag="m")
                nc.tensor.matmul(bc_ps[ds(0, KS), :], ones64[:],
                                 recip2b[:, 0, :], start=True, stop=True)
                nc.tensor.matmul(bc_ps[ds(KS, KS), :], ones64[:],
                                 recip2b[:, 1, :], start=True, stop=True,
                                 tile_position=(0, KS))
                nc.vector.tensor_mul(
                    OT[ds(0, KS), m, ts(ig, IT)], oe_sb[:],
                    bc_ps[ds(0, KS), :])
                nc.vector.tensor_mul(
                    OT[ds(KS, KS), m, ts(ig, IT)], oo_sb[:],
                    bc_ps[ds(KS, KS), :])

            # ---- due-date filler schedule ----------------------------
            # slot t = (ig*NM + pr)*NJ + jc runs scores(ig, pr, jc).
            # A unit's due slot is strictly before its first consumer's
            # slot, so the in-order PE queue never blocks on it.
            fillers = []  # (due, cost_ns, emit)
            sched_log = []

            def add_unit(due, cost, emits, name="u"):
                # stagger sub-units one slot apart so an overdue unit
                # never dumps its whole matmul chain into a single slot
                for k, e in enumerate(emits):
                    fillers.append((due + k, cost, e, f"{name}.{k}"))

            for c4 in range(4):
                if c4 > 0:
                    add_unit(max(0, 4 * c4 - 4), 2 * MM512, k_subunits(c4, 0),
                             f"K{c4}0")
                add_unit(NJ + 4 * c4 - 8, 2 * MM512, k_subunits(c4, 1),
                         f"K{c4}1")
            for g in range(NI):
                for m in range(NM):
                    if g == 0 and m == 0:
                        continue
                    add_unit(max(0, (2 * g + m) * NJ - 12), 2 * MM512,
                             q_subunits(g, m), f"Q{g}{m}")
            for jt in range(NJ):
                add_unit(jt + 2, 2 * MM256, v_subunits(jt), f"V{jt}")
            fillers.sort(key=lambda u: u[0])

            # Wo units of i-group g enter the schedule once both den
            # chains of g have run (their OT inputs are then in flight).
            def add_wo(g, base_slot):
                # dues land in jc 9-14 of the pair: after the (now
                # 4-slot-deferred) den chain's muls have written OT, and
                # outside the boundary hold window
                base = (base_slot // NJ) * NJ + 9
                for idx, (itl, ncol) in enumerate(
                        (i, n) for i in range(NI) for n in range(NO)):
                    fillers.append((max(base + (3 * idx) // 4,
                                        base_slot + 2),
                                    2 * MM512 + 60,
                                    wo_unit(4 * g + itl, ncol),
                                    f"W{g}.{itl}{ncol}"))
                fillers.sort(key=lambda u: u[0])

            def wo3a_unit(itl, ncol):
                """hc=0 half of the last i-group's Wo for row block itl,
                pre-staged to SBUF f32 so the tail only accumulates hc=1."""
                def emit():
                    it = 4 * (NI - 1) + itl
                    w_ps = ps_m.tile([P, 512], F32, tag="m")
                    nc.tensor.matmul(w_ps[:], OT[:, 0, ts(it, P)],
                                     wo_sb[:, 0, ts(ncol, 512)],
                                     start=True, stop=True)
                    nc.vector.tensor_copy(stage3[:, itl, ts(ncol, 512)],
                                          w_ps[:])
                return emit

            def add_wo3a(base_slot):
                for idx, (itl, ncol) in enumerate(
                        (i, n) for i in range(NI) for n in range(NO)):
                    fillers.append((base_slot + 2 + idx, MM512 + 60,
                                    wo3a_unit(itl, ncol), f"W3a.{itl}{ncol}"))
                fillers.sort(key=lambda u: u[0])

            # ---- preamble: the two units slot 0 needs ----------------
            # enough junk matmuls to keep the PE (and its HAM clock) busy
            # until the first wk/wq/xc0 DMAs land, so the preamble
            # projections run at 2.4 GHz instead of the cold 1.2
            wu_ps = ps_m.tile([P, IT], F32, tag="m")
            for _ in range(16):
                nc.tensor.matmul(wu_ps[:], wu_l[:], wu_r[:],
                                 start=True, stop=True)
            wu_sb = cp.tile([1, 1], F32)
            nc.vector.tensor_copy(wu_sb[:], wu_ps[ds(0, 1), ds(0, 1)])
            # interleave K/Q sub-units to match half-chunk DMA arrival
            k00 = k_subunits(0, 0)
            q00 = q_subunits(0, 0)
            for e in (k00[0], k00[1], q00[0], q00[1],
                      k00[2], k00[3], q00[2], q00[3]):
                e()

            # ---- attention loop with filler pump ---------------------
            # The two heads of a pair occupy row groups 0-63 / 64-127, so
            # their score matmuls run CONCURRENTLY in the PE array (row
            # tiling) and one Exp covers both heads' scores.
            o_tiles = {}
            chains_emitted = [0] * NI

            def emit_av(pig, pm, pjc, ppt):
                for par in range(2):
                    key = (pig, 2 * pm + par)
                    if key not in o_tiles:
                        o_tiles[key] = ps_o.tile([KS + 1, IT], F32,
                                                 name="o_ps", tag="o")
                    nc.tensor.matmul(
                        o_tiles[key][:], V_aug[:, 2 * pm + par, pjc, :],
                        ppt[:, par, :],
                        start=(pjc == 0), stop=(pjc == NJ - 1),
                    )
                if pjc == NJ - 1:
                    # evacuate both heads to SBUF (fast PSUM release);
                    # the reciprocal runs NOW on VectorE so the deferred
                    # broadcast-matmul stage never waits on it.  For the
                    # final pair (emitted in the drain) the copies go to
                    # the then-idle ScalarE so the chain never queues
                    # behind VectorE work.
                    last = (pig == NI - 1 and pm == NM - 1)

                    def cpy(dst, src):
                        if last:
                            nc.scalar.copy(dst, src)
                        else:
                            nc.vector.tensor_copy(dst, src)

                    den2 = den_pool.tile([1, 2, IT], F32)
                    evs = []
                    for par in range(2):
                        o_full = o_tiles.pop((pig, 2 * pm + par))
                        o_sb = den_pool.tile([KS, IT], F32, name="o_evac",
                                             tag=f"oev{par}", bufs=3)
                        cpy(o_sb[:], o_full[ds(0, KS), :])
                        cpy(den2[:, par, :], o_full[ds(KS, 1), :])
                        evs.append(o_sb)
                    recip2 = den_pool.tile([1, 2, IT], F32)
                    nc.vector.reciprocal_approx_fast(
                        recip2[:].rearrange("p a b -> p (a b)"),
                        den2[:].rearrange("p a b -> p (a b)"))
                    recip2b = den_pool.tile([1, 2, IT], MMDT)
                    nc.vector.tensor_copy(
                        recip2b[:].rearrange("p a b -> p (a b)"),
                        recip2[:].rearrange("p a b -> p (a b)"))
                    # defer 4 slots (with 3 den-pool bufs to cover the
                    # longer tile lifetime): the evac+recip+cast chain on
                    # VectorE takes ~3.4us and the broadcast matmul must
                    # not reach the in-order PE queue head before recip2b
                    # is ready, or it stalls the scores behind it
                    chain_q.append([4, (pig, pm, evs[0], evs[1], recip2b)])

            pending = []  # (ig, m, jc, pt), AV emitted at depth >= lag
            chain_q = []  # deferred pair normalization chains
            for ig in range(NI):
                for pr in range(NM):
                    for jc in range(NJ):
                        t = (ig * NM + pr) * NJ + jc
                        sP = ps_s.tile([P, NJJ, IT], F32, tag="s")
                        nc.tensor.matmul(
                            sP[:, 0, :],
                            KT[ds(0, KS), pr, ts(jc, P)],
                            QT[ds(0, KS), pr, ts(ig, IT)],
                            start=True, stop=True,
                        )
                        nc.tensor.matmul(
                            sP[:, 1, :],
                            KT[ds(KS, KS), pr, ts(jc, P)],
                            QT[ds(KS, KS), pr, ts(ig, IT)],
                            start=True, stop=True,
                        )
                        tail6 = (ig == NI - 1 and pr == NM - 1
                                 and jc >= NJ - 6)
                        lag = 2 if tail6 else 4
                        used = 0
                        # drain a pair's tail AVs at 2/slot and hold the
                        # next pair's first AV until jc 6, so the PSUM
                        # evacuation copy of pair N lands several slots
                        # before pair N+1's first AV needs its bank
                        npop = 2 if (tail6 or len(pending) > 9
                                     or (pending and pending[0][2] >= NJ - 11)
                                     ) else 1
                        for _ in range(npop):
                            if (len(pending) >= lag and v_done[pending[0][2]]
                                    and not (pending[0][2] == 0 and jc < 7
                                             and not tail6)):
                                emit_av(*pending.pop(0))
                                used += 2 * MM512
                        if chain_q:
                            chain_q[0][0] -= 1
                            if chain_q[0][0] <= 0:
                                ent = chain_q.pop(0)[1]
                                den_chain(*ent)
                                used += MM512
                                cg = ent[0]
                                chains_emitted[cg] += 1
                                if chains_emitted[cg] == NM and cg < NI - 1:
                                    add_wo(cg, t)
                                if cg == NI - 1 and chains_emitted[cg] == 1:
                                    add_wo3a(t)
                        budget = 700 if used > 400 else 1100
                        while fillers and (fillers[0][0] <= t
                                           or used + fillers[0][1] <= budget):
                            _, cost, e, nm = fillers.pop(0)
                            e()
                            used += cost
                            sched_log.append((t, nm, used))
                        pt = probs_pool.tile([P, NJJ, IT], MMDT)
                        nc.scalar.activation(
                            pt[:].rearrange("p a b -> p (a b)"),
                            sP[:].rearrange("p a b -> p (a b)"),
                            EXP, scale=0.125)
                        pending.append((ig, pr, jc, pt))
            # drain; junk matmuls keep the PE warm (and the HAM clock at
            # 2.4 GHz) through the last pair's evac + recip + chain, so
            # the tail Wo matmuls don't run at the cold 1.2 GHz
            wu_ps2 = ps_s.tile([P, NJJ, IT], F32, name="wu_ps2", tag="s")
            emit_av(*pending.pop(0))
            for _ in range(4):
                nc.tensor.matmul(wu_ps2[:, 0, :], wu_l[:], wu_r[:],
                                 start=True, stop=True)
            emit_av(*pending.pop(0))
            assert not pending
            # ~14 junk matmuls ~= the DVE latency of the last pair's
            # evac + recip chain: they keep the HAM clock warm without
            # delaying the broadcast matmuls behind them in the PE queue
            # (the bc matmul waits on the reciprocal cast anyway)
            for _ in range(14):
                nc.tensor.matmul(wu_ps2[:, 0, :], wu_l[:], wu_r[:],
                                 start=True, stop=True)
            while chain_q:
                den_chain(*chain_q.pop(0)[1])
            for _ in range(4):
                nc.tensor.matmul(wu_ps2[:, 0, :], wu_l[:], wu_r[:],
                                 start=True, stop=True)
            nc.vector.tensor_copy(wu_sb[:], wu_ps2[ds(0, 1), 0, ds(0, 1)])
            for _, _, e, _nm in fillers:
                e()
            if os.environ.get("SCHED_DEBUG"):
                from collections import defaultdict
                per = defaultdict(list)
                for t_, nm, u_ in sched_log:
                    per[t_].append((nm, u_))
                for t_ in sorted(per):
                    print(t_, per[t_])

            # tail: Wo of the last i-group -- only the hc=1 matmuls (the
            # hc=0 half sits pre-staged in stage3), fused add -> bf16
            # and a 128 KB DMA per 512-col chunk for a fine pipeline
            for k, (itl, ncol) in enumerate(
                    (i, n) for i in range(NI) for n in range(NO)):
                it = 4 * (NI - 1) + itl
                # rotate across the freed score AND o-accumulator tags so
                # the PE runs ahead of the DVE add cascade (4-deep)
                pool, tg = (ps_s, "s") if k % 2 == 0 else (ps_o, "o")
                wps = pool.tile([P, 512], F32, name="wops", tag=tg)
                nc.tensor.matmul(
                    wps[:],
                    OT[:, 1, ts(it, P)],
                    wo_sb[:, 1, ts(ncol, 512)],
                    start=True, stop=True,
                )
                o_sb = out_pool.tile([P, 512], MMDT, name="o_tail",
                                     tag="otl", bufs=4)
                nc.vector.tensor_add(o_sb[:],
                                     stage3[:, itl, ts(ncol, 512)],
                                     wps[:])
                # all three DMA queues are idle in the tail -- rotate
                # across them so the final flush transfers run parallel
                eng = (nc.sync, nc.gpsimd, nc.scalar)[k % 3]
                eng.dma_start(out[ts(it, P), ts(ncol, 512)], o_sb[:])


def _build_program():
    nc = bacc.Bacc("TRN2", target_bir_lowering=False, debug=False,
                   num_devices=NCORES)
    # all inputs come pre-arranged from the host in SBUF layout
    # (partition-major, fully contiguous per partition line)
    xT = nc.dram_tensor("xT", (P, 4 * ND * IT), MMDT,
                        kind="ExternalInput").ap()
    wkq = nc.dram_tensor("wkq", (P, 2 * ND * GW + ND * IT), MMDT,
                         kind="ExternalInput").ap()
    wv = nc.dram_tensor("wv", (P, ND * GW), MMDT, kind="ExternalInput").ap()
    wo = nc.dram_tensor("wo", (P, NM * D), MMDT, kind="ExternalInput").ap()
    bq = nc.dram_tensor("bq", (P, NM), F32, kind="ExternalInput").ap()
    bk = nc.dram_tensor("bk", (P, NM), F32, kind="ExternalInput").ap()
    bv = nc.dram_tensor("bv", (GW,), F32, kind="ExternalInput").ap()
    out = nc.dram_tensor("out", (S, D), MMDT, kind="ExternalOutput").ap()
    with tile.TileContext(nc) as tc:
        _mha_core(tc, out, xT, wkq, wv, wo, bq, bk, bv)
    nc.compile()
    return nc


_program = None


def _get_program():
    global _program
    if _program is None:
        _program = _build_program()
    return _program


def _w_pre(W, bf):
    """(D, GW) -> SBUF layout (P, ND*GW): [p, nd*GW+n] = W[nd*P+p, n]."""
    return np.ascontiguousarray(
        W.reshape(ND, P, GW).transpose(1, 0, 2).reshape(P, ND * GW)
    ).astype(bf)


def make_in_maps(x, Wq, bq, Wk, bk, Wv, bv, Wo, bo):
    in_maps = []
    f = np.float32
    bf = mybir.dt.np(MMDT)
    for c in range(NCORES):
        b, g = divmod(c, 4)
        sl = slice(g * GW, (g + 1) * GW)
        # x[b] is (S, D); SBUF layout (P, 4, ND, IT):
        # [p, c4, nd, s] = x[b][c4*IT+s, nd*P+p]
        xpre = np.asarray(x[b], dtype=np.float32).reshape(4, IT, ND, P)
        xpre = xpre.transpose(3, 0, 2, 1).reshape(P, 4 * ND * IT)
        xpre = np.ascontiguousarray(xpre).astype(bf)
        wopre = np.asarray(Wo[sl, :]).reshape(NM, P, D).transpose(1, 0, 2)
        wkq = np.concatenate(
            [_w_pre(np.asarray(Wk[:, sl]), bf),
             _w_pre(np.asarray(Wq[:, sl]), bf),
             xpre[:, :ND * IT]], axis=1)
        in_maps.append({
            "xT": xpre,
            "wkq": np.ascontiguousarray(wkq),
            "wv": _w_pre(np.asarray(Wv[:, sl]), bf),
            "wo": np.ascontiguousarray(wopre.reshape(P, NM * D)).astype(bf),
            "bq": np.ascontiguousarray(
                np.asarray(bq[sl], dtype=f).reshape(NM, P).T),
            "bk": np.ascontiguousarray(
                np.asarray(bk[sl], dtype=f).reshape(NM, P).T),
            "bv": np.ascontiguousarray(bv[sl], dtype=f),
        })
    return in_maps


def run(inputs, trace=False, tmpdir=None, **kw):
    nc = _get_program()
    in_maps = make_in_maps(**inputs)
    res = run_bass_kernel_spmd(nc, in_maps, core_ids=list(range(NCORES)),
                               trace=trace, tmpdir=tmpdir, **kw)
    bo = inputs["bo"].astype(np.float32)
    parts = [np.asarray(res.results[c]["out"], dtype=np.float32)
             for c in range(NCORES)]
    y = np.stack(
        [parts[4 * b] + parts[4 * b + 1] + parts[4 * b + 2] + parts[4 * b + 3] + bo
         for b in range(B)], axis=0)
    return y.astype(np.float32), res


def kernel(**inputs):
    y, _ = run(inputs, trace=False)
    return y
